# revision 2
# baseline (speedup 1.0000x reference)
"""GQA attention kernel for Trainium2, sharded over 8 NeuronCores.

Sharding: 2-way batch data-parallel x 4-way kv-head tensor-parallel.
Core c = (b, k) with b = c // 4, k = c % 4 owns batch b, kv-heads
{2k, 2k+1}, q-heads {8k..8k+7}, and o_proj contraction features
512k:512k+512. Each core emits a [S, H] bf16 partial; the host sums the
4 partials per batch and adds bo.

Per-core layout:
- hsT [H, S] so projections contract over the partition dim.
- Q^T tiles: tile i holds local q-head i at partitions 0:64 and head
  4+i at 64:128, so every q-head sits at the same partition base as its
  kv head (kv0 at 0:64, kv1 at 64:128 of kT) - no K replication needed.
- Scores are computed transposed S^T[t, q] = K^T . Q so the softmax
  mask+scale fold into the exp activation (bias is per-partition) and
  one ACT instruction covers a head-pair's [128, 2, 512] scores tile.
- Context uses the exp tile as the matmul *stationary* and V|1 [128, 65]
  as the moving operand: out ctx[q, 65] gets full 128 output partitions,
  128-deep contraction, and only 65 moving columns per (q-tile, t-tile);
  column 64 accumulates the softmax denominators.
- Everything streams bf16 (PSUM accumulation stays fp32).
"""

import os
import sys

for _p in ("/opt/trn_rl_repo",):
    if _p not in sys.path and os.path.isdir(_p):
        sys.path.insert(0, _p)

import numpy as np
import ml_dtypes

import concourse.bass as bass
import concourse.bacc as bacc
import concourse.tile as tile
from concourse import mybir
from concourse import bass_utils

F32 = mybir.dt.float32
F32R = mybir.dt.float32r
BF16 = mybir.dt.bfloat16
AF = mybir.ActivationFunctionType

B = 2
S = 2048
H = 2048
D = 64
N_CORES = 8
QH = 8                 # local q-heads per core
KVH = 2                # local kv-heads per core
QF = QH * D            # 512 q features per core
NF = QF + 2 * KVH * D  # 768 projected features per core
CHUNK = 512            # q-token chunk
NCHUNK = S // CHUNK    # 4
SCALE = 1.0 / np.sqrt(D)

LAG = int(os.environ.get("K2_LAG", "2"))
W_EARLY = float(os.environ.get("K2_W_EARLY", "1.0"))
W_LATE = float(os.environ.get("K2_W_LATE", "1.0"))
W_EDGE = float(os.environ.get("K2_W_EDGE", "1.0"))
EX_BUFS = int(os.environ.get("K2_EX_BUFS", "6"))

_CACHE = {}


def _build_program():
    nc = bacc.Bacc("TRN2", target_bir_lowering=False, debug=False)

    hsT = nc.dram_tensor("hsT", [H, S], BF16, kind="ExternalInput").ap()
    wqkv = nc.dram_tensor("wqkv", [H, NF], BF16, kind="ExternalInput").ap()
    woT = nc.dram_tensor("woT", [QF, H], BF16, kind="ExternalInput").ap()
    bqkv = nc.dram_tensor("bqkv", [128, 6], F32, kind="ExternalInput").ap()
    maskp = nc.dram_tensor("maskp", [128, S // 128], F32, kind="ExternalInput").ap()
    eyeq = nc.dram_tensor("eyeq", [128, 64], F32R, kind="ExternalInput").ap()
    eyef = nc.dram_tensor("eyef", [128, 128], F32R, kind="ExternalInput").ap()
    out = nc.dram_tensor("out", [S, H], BF16, kind="ExternalOutput").ap()

    NT = S // 128  # 16 key tiles

    with tile.TileContext(nc) as tc:
        with tc.tile_pool(name="const", bufs=1) as cp:
            # K weight columns first: the first projection matmuls need only
            # them, so the startup critical path skips every other load.
            w_qkv = cp.tile([128, 16, NF], BF16)      # (p, h_tile, feature)
            wqkv_t = wqkv.rearrange("(t p) f -> p t f", p=128)
            # ht-major halves: >=512B runs, and the first 8 contraction tiles
            # can start after half the weight + half the hstage transfer
            nc.sync.dma_start(out=w_qkv[:, 0:8, QF:NF],
                              in_=wqkv_t[:, 0:8, QF:NF])
            bqkv_sb = cp.tile([128, 6], F32)
            mask_sb = cp.tile([128, NT], F32)
            eyeq_sb = cp.tile([128, 64], F32R)
            eyef_sb = cp.tile([128, 128], F32R)
            w_o = cp.tile([128, 4, H], BF16)          # (p, f_tile, e)
            wo_t = woT.rearrange("(t p) e -> p t e", p=128)

            kT = cp.tile([128, S], BF16)              # kv0 @ 0:64, kv1 @ 64:128
            vT32 = cp.tile([128, S], F32R)            # same partition layout
            qT = cp.tile([128, 4, S], BF16)           # tile i: head i | head 4+i
            vones = cp.tile([128, NT, 2, 65], BF16)   # (t_p, t_tile, kv, d|1)

            hsT_tiled = hsT.rearrange("(t p) n -> p t n", p=128)

            with tc.tile_pool(name="hs_sb", bufs=4) as hsb, \
                 tc.tile_pool(name="work_ps", bufs=2, space="PSUM") as wps, \
                 tc.tile_pool(name="sc_ps", bufs=2, space="PSUM") as sps, \
                 tc.tile_pool(name="ctx_ps", bufs=2, space="PSUM") as xps, \
                 tc.tile_pool(name="ex_sb", bufs=EX_BUFS) as esb, \
                 tc.tile_pool(name="drain_sb", bufs=3) as dsb, \
                 tc.tile_pool(name="ctxn_sb", bufs=2) as nsb, \
                 tc.tile_pool(name="ctxT_sb", bufs=2) as csb, \
                 tc.tile_pool(name="out_sb", bufs=3) as osb_pool:

                # ---- build V[t, d] (+mask col) via PE transpose ----
                # The attention mask enters here, not in the exp: softmax of
                # (S + m[t]) equals exp(S) weighted per key by exp(m[t]), and
                # t is the partition dim of vones. Column 64 holds exp(m[t])
                # so the denominator matmul applies the same weights.
                expm_sb = cp.tile([128, NT], F32)

                def vones_tile(t):
                    for j in range(2):
                        tp = wps.tile([128, 512], F32, tag="work",
                                      name=f"tp_{t}_{j}")
                        nc.tensor.transpose(
                            tp[:, 0:64].bitcast(F32R),
                            in_=vT32[j * 64:(j + 1) * 64, t * 128:(t + 1) * 128],
                            identity=eyeq_sb[j * 64:(j + 1) * 64, :])
                        nc.vector.tensor_scalar_mul(
                            out=vones[:, t, j, 0:64], in0=tp[:, 0:64],
                            scalar1=expm_sb[:, t:t + 1])

                # ---- K/V projection over all tokens ----
                WQ_AT = {2: 0, 4: 1}  # ck -> 256-col Q-weight slice DMA
                for ck in range(S // 256):
                    hstage = hsb.tile([128, 16, 256], BF16, tag="hstage")
                    nc.sync.dma_start(out=hstage,
                                      in_=hsT_tiled[:, :, ck * 256:(ck + 1) * 256])
                    if ck == 0:
                        # second KV-weight half + small consts: needed a few
                        # µs in; issued after hstage0 so its transfer goes
                        # first.
                        nc.sync.dma_start(out=w_qkv[:, 8:16, QF:NF],
                                          in_=wqkv_t[:, 8:16, QF:NF])
                        nc.sync.dma_start(out=bqkv_sb, in_=bqkv)
                        nc.sync.dma_start(out=eyeq_sb, in_=eyeq)
                        nc.sync.dma_start(out=mask_sb, in_=maskp)
                        nc.sync.dma_start(out=eyef_sb, in_=eyef)
                        nc.scalar.activation(out=expm_sb, in_=mask_sb,
                                             func=AF.Exp)
                        for j in range(2):
                            nc.vector.tensor_copy(out=vones[:, :, j, 64],
                                                  in_=expm_sb)
                    if ck in WQ_AT:
                        # Q weights: needed after the KV pass; issued in
                        # >=512B slices (smaller runs pay a 2x DMA penalty)
                        # mid-stream so hstage transfers are never delayed.
                        ft = WQ_AT[ck]
                        nc.sync.dma_start(
                            out=w_qkv[:, :, ft * 256:(ft + 1) * 256],
                            in_=wqkv_t[:, :, ft * 256:(ft + 1) * 256])

                    for ft in (4, 5):
                        ps = wps.tile([128, 512], F32, tag="work")
                        for ht in range(16):
                            nc.tensor.matmul(
                                ps[:, 0:256],
                                w_qkv[:, ht, ft * 128:(ft + 1) * 128],
                                hstage[:, ht, :],
                                start=(ht == 0), stop=(ht == 15),
                            )
                        if ft == 4:
                            nc.scalar.activation(
                                out=kT[:, ck * 256:(ck + 1) * 256], in_=ps[:, 0:256],
                                func=AF.Identity, bias=bqkv_sb[:, 4:5])
                        else:
                            nc.scalar.activation(
                                out=vT32[:, ck * 256:(ck + 1) * 256], in_=ps[:, 0:256],
                                func=AF.Identity, bias=bqkv_sb[:, 5:6])
                    # transpose V tiles one chunk behind the V drains
                    if ck >= 1:
                        vones_tile(2 * (ck - 1))
                        vones_tile(2 * (ck - 1) + 1)
                vones_tile(14)
                vones_tile(15)

                # ---- chunk loop: Q-proj + attention + o_proj pipelined ----
                ctxT_of = {}

                def qproj_slice(c, ck2, ft, hq, ps):
                    tk0 = c * CHUNK + ck2 * 256
                    for ht in range(4 * hq, 4 * hq + 4):
                        nc.tensor.matmul(
                            ps[:, 0:256],
                            w_qkv[:, ht, ft * 128:(ft + 1) * 128],
                            hstage_of[(c, ck2)][:, ht, :],
                            start=(ht == 0), stop=(ht == 15),
                        )
                    if hq == 3:
                        nc.vector.tensor_scalar_add(
                            out=qT[:, ft, tk0:tk0 + 256], in0=ps[:, 0:256],
                            scalar1=bqkv_sb[:, ft:ft + 1])

                def oproj_slice(c, tt, ec, ft, osb, op):
                    ctxT = ctxT_of[c]
                    nc.tensor.matmul(
                        op,
                        ctxT[:, ft, tt * 128:(tt + 1) * 128],
                        w_o[:, ft, ec * 512:(ec + 1) * 512],
                        start=(ft == 0), stop=(ft == 3),
                    )
                    if ft == 3:
                        nc.vector.tensor_copy(
                            out=osb[:, ec * 512:(ec + 1) * 512], in_=op)
                        if ec == 3:
                            nc.gpsimd.dma_start(
                                out=out[c * CHUNK + tt * 128:
                                        c * CHUNK + (tt + 1) * 128, :],
                                in_=osb)

                def oproj_fills(c):
                    for tt in range(CHUNK // 128):
                        osb = osb_pool.tile([128, H], BF16, tag="osb",
                                            name=f"osb_{c}_{tt}")
                        for ec in range(4):
                            op = wps.tile([128, 512], F32, tag="work",
                                          name=f"op_{c}_{tt}_{ec}")
                            for ft in range(0, 4, 2):
                                yield lambda c=c, tt=tt, ec=ec, ft=ft, \
                                    osb=osb, op=op: (
                                        oproj_slice(c, tt, ec, ft, osb, op),
                                        oproj_slice(c, tt, ec, ft + 1, osb, op))

                def qproj_fills(c):
                    for ck2 in range(CHUNK // 256):
                        tk0 = c * CHUNK + ck2 * 256
                        hstage = hsb.tile([128, 16, 256], BF16, tag="hstage",
                                          name=f"hs_{c}_{ck2}")
                        hstage_of[(c, ck2)] = hstage
                        nc.sync.dma_start(out=hstage,
                                          in_=hsT_tiled[:, :, tk0:tk0 + 256])
                        for ft in range(4):
                            ps = wps.tile([128, 512], F32, tag="work",
                                          name=f"qp_{c}_{ck2}_{ft}")
                            for hq in range(4):
                                yield lambda c=c, ck2=ck2, ft=ft, hq=hq, \
                                    ps=ps: qproj_slice(c, ck2, ft, hq, ps)

                hstage_of = {}
                # Q-proj for chunk 0 runs up front (dense PE work, no fills)
                for fill in qproj_fills(0):
                    fill()
                # o_proj weights: first needed mid-chunk-1, queued after the
                # chunk-0 activations stream
                for ft in range(4):
                    nc.sync.dma_start(out=w_o[:, ft, :], in_=wo_t[:, ft, :])

                for c in range(NCHUNK):
                    q0 = c * CHUNK
                    fills = []
                    if c > 0:
                        fills.extend(oproj_fills(c - 1))
                    if c + 1 < NCHUNK:
                        fills.extend(qproj_fills(c + 1))
                    # spread fills across the (pair, t) iterations, weighted
                    # by each iteration's PE deficit (t<LAG has no ctx work,
                    # t>=NT has no scores), saving a little for the
                    # chunk-boundary drain bubble
                    wts = []
                    for _p in range(4):
                        for _t in range(NT + LAG):
                            wts.append(W_EARLY if _t < LAG else
                                       (W_LATE if _t >= NT else 1.0))
                    wts.append(W_EDGE)  # boundary reserve
                    cum = np.cumsum(wts) / sum(wts)
                    fill_idx = [0]

                    def pop_fill(it):
                        quota = int(round(cum[it] * len(fills)))
                        while fill_idx[0] < quota:
                            fills[fill_idx[0]]()
                            fill_idx[0] += 1

                    ctxn = nsb.tile([128, 4, QH, 64], F32R, tag="ctxn",
                                    name=f"ctxn_{c}")
                    ctxT = csb.tile([128, 4, CHUNK], BF16, tag="ctxT",
                                    name=f"ctxT_{c}")
                    ctxT_of[c] = ctxT

                    def pair_finish_dve(c, p, ctxs, ctxn):
                        # drain + normalize the pair; the cs copy releases the
                        # pair's PSUM banks, so it must lead the DVE queue
                        h0 = 2 * p
                        for i, h in enumerate((h0, h0 + 1)):
                            cs = dsb.tile([128, 4, 65], F32, tag="cs",
                                          name=f"cs_{c}_{p}_{i}")
                            nc.vector.tensor_copy(out=cs, in_=ctxs[i])
                            rc = dsb.tile([128, 4, 1], F32, tag="rc",
                                          name=f"rc_{c}_{p}_{i}")
                            nc.vector.reciprocal(out=rc, in_=cs[:, :, 64:65])
                            for qt in range(4):
                                nc.vector.tensor_scalar_mul(
                                    out=ctxn[:, qt, h, :], in0=cs[:, qt, 0:64],
                                    scalar1=rc[:, qt, :])

                    def pair_finish_pe(c, p, ctxn, ctxT):
                        # transpose the pair's [q, f] block into ctxT [f, q]
                        h0 = 2 * p
                        for qt in range(4):
                            tp2 = wps.tile([128, 512], F32, tag="work",
                                           name=f"tp2_{c}_{p}_{qt}")
                            nc.tensor.transpose(
                                tp2[:, 0:128].bitcast(F32R),
                                in_=ctxn[:, qt, h0:h0 + 2, :],
                                identity=eyef_sb)
                            nc.vector.tensor_copy(
                                out=ctxT[:, p, qt * 128:(qt + 1) * 128],
                                in_=tp2[:, 0:128])

                    it = 0
                    finish_prev = [None]
                    for p in range(4):           # head pairs (2p, 2p+1)
                        h0, h1 = 2 * p, 2 * p + 1
                        base = 64 * (h0 // 4)    # shared kv partition base
                        kv = h0 // 4
                        ctxs = [xps.tile([128, 4, 65], F32, tag="ctx",
                                         name=f"ctx_{c}_{p}_{i}")
                                for i in range(2)]
                        exs = {}
                        for t in range(NT + LAG):
                            if t < NT:
                                sc = sps.tile([128, 2, 512], F32, tag="sc",
                                              name=f"sc_{c}_{p}_{t}")
                                for i, h in enumerate((h0, h1)):
                                    nc.tensor.matmul(
                                        sc[:, i, :],
                                        kT[base:base + 64,
                                           t * 128:(t + 1) * 128],
                                        qT[base:base + 64, h % 4,
                                           q0:q0 + CHUNK],
                                        start=True, stop=True,
                                    )
                                ex = esb.tile([128, 2, 512], BF16, tag="ex",
                                              name=f"ex_{c}_{p}_{t}")
                                nc.scalar.activation(
                                    out=ex, in_=sc, func=AF.Exp, scale=SCALE)
                                exs[t] = ex
                            if t >= LAG:
                                ex = exs.pop(t - LAG)
                                for i in range(2):
                                    for qt in range(4):
                                        # start zeroes the whole 2KB bank, so
                                        # only the bank's first matmul starts
                                        # and only its last one stops
                                        nc.tensor.matmul(
                                            ctxs[i][:, qt, :],
                                            ex[:, i, qt * 128:(qt + 1) * 128],
                                            vones[:, t - LAG, kv, :],
                                            start=(t - LAG == 0 and qt == 0),
                                            stop=(t - LAG == NT - 1 and qt == 3),
                                        )
                            if t == 0 and finish_prev[0] is not None:
                                # previous pair's DVE drain leads the queue so
                                # its PSUM banks free before fills enqueue
                                finish_prev[0][0]()
                            if t == 3 and finish_prev[0] is not None:
                                # its PE transposes go later, once the DVE
                                # chain has surely finished
                                finish_prev[0][1]()
                                finish_prev[0] = None
                            pop_fill(it)
                            it += 1
                        finish_prev[0] = (
                            lambda c=c, p=p, ctxs=ctxs:
                            pair_finish_dve(c, p, ctxs, ctxn),
                            lambda c=c, p=p:
                            pair_finish_pe(c, p, ctxn, ctxT),
                        )
                    finish_prev[0][0]()
                    finish_prev[0][1]()
                    while fill_idx[0] < len(fills):
                        fills[fill_idx[0]]()
                        fill_idx[0] += 1

                # tail: o_proj of the last chunk
                for fill in oproj_fills(NCHUNK - 1):
                    fill()
    nc.compile()
    return nc


def kernel(hidden_states, attention_mask, Wq, bq, Wk, bk, Wv, bv, Wo, bo):
    hidden_states = np.asarray(hidden_states, dtype=np.float32)
    attention_mask = np.asarray(attention_mask, dtype=np.float32)
    Wq = np.asarray(Wq, dtype=np.float32)
    Wk = np.asarray(Wk, dtype=np.float32)
    Wv = np.asarray(Wv, dtype=np.float32)
    Wo = np.asarray(Wo, dtype=np.float32)
    bq = np.asarray(bq, dtype=np.float32)
    bk = np.asarray(bk, dtype=np.float32)
    bv = np.asarray(bv, dtype=np.float32)

    if "nc" not in _CACHE:
        _CACHE["nc"] = _build_program()
    nc = _CACHE["nc"]

    bf = ml_dtypes.bfloat16
    eyeq = np.zeros((128, 64), dtype=np.float32)
    eyeq[0:64] = np.eye(64, dtype=np.float32)
    eyeq[64:128] = np.eye(64, dtype=np.float32)
    eyef = np.eye(128, dtype=np.float32)

    in_maps = []
    for core in range(N_CORES):
        b, k = divmod(core, 4)
        qrows = []
        for i in range(4):
            qrows += list(range((8 * k + i) * 64, (8 * k + i) * 64 + 64))
            qrows += list(range((8 * k + 4 + i) * 64, (8 * k + 4 + i) * 64 + 64))
        qrows = np.array(qrows)
        wq_p = Wq[qrows]                       # [512, H]
        wk_p = Wk[128 * k:128 * (k + 1)]       # [128, H]
        wv_p = Wv[128 * k:128 * (k + 1)]
        wqkv_np = np.ascontiguousarray(
            np.concatenate([wq_p, wk_p, wv_p], axis=0).T).astype(bf)  # [H, 768]
        woT_np = np.ascontiguousarray(
            Wo[:, 512 * k:512 * (k + 1)].T).astype(bf)                # [512, H]
        bq_p = bq[qrows].reshape(4, 128)
        bkv = np.concatenate([bk[128 * k:128 * (k + 1)],
                              bv[128 * k:128 * (k + 1)]]).reshape(2, 128)
        bqkv_np = np.ascontiguousarray(
            np.concatenate([bq_p, bkv], axis=0).T).astype(np.float32)  # [128, 6]
        hsT_np = np.ascontiguousarray(hidden_states[b].T).astype(bf)   # [H, S]
        maskp_np = np.ascontiguousarray(
            attention_mask[b].reshape(S // 128, 128).T).astype(np.float32)
        in_maps.append({
            "hsT": hsT_np, "wqkv": wqkv_np, "woT": woT_np,
            "bqkv": bqkv_np, "maskp": maskp_np, "eyeq": eyeq, "eyef": eyef,
        })

    _CACHE["last_in_maps"] = in_maps
    res = bass_utils.run_bass_kernel_spmd(nc, in_maps, core_ids=list(range(N_CORES)))
    acc = np.zeros((B, S, H), dtype=np.float32)
    for core in range(N_CORES):
        b = core // 4
        acc[b] += np.asarray(res.results[core]["out"], dtype=np.float32)
    acc += np.asarray(bo, dtype=np.float32)[None, None, :]
    return acc
